# revision 8
# baseline (speedup 1.0000x reference)
"""Trainium2 Bass kernel for nn_DendriteOutput.

Math: out[b, o] = sum_{d<32} x[b, o*32+d] * weight[o, o*32+d] + bias[o]
(block-diagonal connectivity: only the diagonal 32-wide blocks of `weight`
are touched, so the kernel never reads the other 99.2% of the matrix).

Sharding (8 cores, tensor-parallel over out_dim):
  core k handles outputs [k*256, (k+1)*256) for the full batch, i.e. the
  x column-slab [:, k*8192:(k+1)*8192] (32 MB/core -> the dominant HBM
  traffic; per-core roofline ~ 33 MB / ~358 GB/s ~ 92 us).

Per-core pipeline (batch tiles of 128 rows = SBUF partitions):
  1. HWDGE DMA the x tile [128, 8192] f32 (measured ~368 GB/s; the SWDGE
     cast-in-DMA path was ~30% slower, so casts stay on engines).
  2. Cast f32 -> fp16: N_ACT tiles on ScalarE, the rest on DVE
     (tensor_copy, 2x_2P mode) so neither engine exceeds the ~87 us
     DMA stream time.
  3. DVE: elementwise multiply with the fp16 partition-replicated diagonal
     weight strip (host-prepared, plain 2 MB contiguous load), then a
     log-tree segmented reduction 32->16->8->4->2->1 (strided tensor_adds
     in fp16 2x mode; the last level + bias add in fp32).
  4. HWDGE DMA the [128, 256] f32 output tile out.
"""

import json

import numpy as np

import concourse.bass as bass
import concourse.bass_utils as _bass_utils
import concourse.mybir as mybir
from concourse.tile import TileContext
from concourse.bass_utils import run_bass_kernel_spmd

BATCH = 1024
OUT_DIM = 2048
DPC = 32
N_CORES = 8
O_PER = OUT_DIM // N_CORES          # 256 outputs per core
F_PER = O_PER * DPC                 # 8192 features per core
BT = 128                            # batch rows per tile (SBUF partitions)
N_BT = BATCH // BT                  # 8 batch tiles per core
N_ACT = 2                           # tiles whose f32->fp16 cast runs on ScalarE

# ---------------------------------------------------------------------------
# Environment workarounds (in-process only; nothing on disk is modified).
#
# The walrus build in this container (a) needs --dge-levels to lower HWDGE
# DMAs with sem waits (otherwise they hit the V2 pseudo-DMA path that allows
# none) and (b) caps sync waits at ONE per instruction while Tile attaches up
# to N (e.g. the kernel-tail drain). We add the flag and rewrite the
# serialized BIR: extra waits are hoisted into preceding single-wait Drain
# carriers on the same engine (safe: a wait only moves earlier within the
# same engine-program order).
# ---------------------------------------------------------------------------

_patched = False


def _patch_walrus_flags():
    global _patched
    if _patched:
        return
    _patched = True
    orig_rc = _bass_utils.run_command

    def rc(cmd, cwd=None, **kw):
        if cmd and "walrus_driver" in str(cmd[0]):
            cmd = list(cmd)
            cmd.insert(1, "--dge-levels=io,spill_reload,scalar_dynamic_offset")
        return orig_rc(cmd, cwd=cwd, **kw)

    _bass_utils.run_command = rc


def _split_multi_waits(bir_bytes: bytes, cap: int = 1) -> bytes:
    m = json.loads(bir_bytes)
    for fn in m["functions"]:
        for blk in fn["blocks"]:
            out = []
            for inst in blk["instructions"]:
                si = inst.get("sync_info")
                waits = (si or {}).get("on_wait") or []
                if len(waits) > cap:
                    keep = waits[-cap:]
                    for j, wchunk in enumerate(waits[:-cap]):
                        out.append(
                            {
                                "debug": inst.get("debug"),
                                "engine": inst["engine"],
                                "ins": [],
                                "name": f"{inst['name']}-ws{j}",
                                "opcode": "Drain",
                                "outs": [],
                                "sync_info": {
                                    "on_update": [],
                                    "on_wait": [wchunk],
                                },
                            }
                        )
                    si["on_wait"] = keep
                out.append(inst)
            blk["instructions"] = out
    return json.dumps(m).encode()


def _emit_body(nc, tc, x, w, b, y, rep=0):
    """Emit one full per-core kernel inside an open TileContext."""
    f32 = mybir.dt.float32
    f16 = mybir.dt.float16
    with (
        tc.tile_pool(name=f"const{rep}", bufs=1) as cpool,
        tc.tile_pool(name=f"work{rep}", bufs=3) as wpool,
        tc.tile_pool(name=f"outp{rep}", bufs=3) as opool,
    ):
        # wrep/brep/stores ride the ACT HWDGE queue; the sync HWDGE queue
        # carries ONLY the 8 x loads so no store-wait ever stalls the
        # load FIFO (HWDGE DMAs are FIFO per issuing engine).
        wrep = cpool.tile([128, F_PER], f16, name=f"wrep{rep}")
        brep = cpool.tile([128, O_PER], f32, name=f"brep{rep}")
        nc.scalar.dma_start(wrep[:], w[:, :])
        nc.scalar.dma_start(brep[:], b[:, :])

        for i in range(N_BT):
            xt32 = wpool.tile([128, F_PER], f32, tag="xt32", bufs=3,
                              name=f"xt32_{rep}_{i}")
            nc.sync.dma_start(xt32[:], x[i * BT : (i + 1) * BT, :])
            xt = wpool.tile([128, F_PER], f16, tag="xt", bufs=2,
                            name=f"xt{rep}_{i}")
            act_tiles = {round(j * N_BT / N_ACT) for j in range(N_ACT)}
            if i in act_tiles:
                nc.scalar.copy(xt[:], xt32[:])
            else:
                nc.vector.tensor_copy(xt[:], xt32[:])
            nc.vector.tensor_mul(xt[:], xt[:], wrep[:])
            p3 = xt[:].rearrange("p (o d) -> p o d", d=DPC)
            q1 = wpool.tile([128, O_PER * 16], f16, tag="q1", bufs=2,
                            name=f"q1_{rep}_{i}")
            q1v = q1[:].rearrange("p (o d) -> p o d", d=16)
            nc.vector.tensor_add(q1v, p3[:, :, 0:16], p3[:, :, 16:32])
            q2 = wpool.tile([128, O_PER * 8], f16, tag="q2", bufs=2,
                            name=f"q2_{rep}_{i}")
            q2v = q2[:].rearrange("p (o d) -> p o d", d=8)
            nc.vector.tensor_add(q2v, q1v[:, :, 0:8], q1v[:, :, 8:16])
            q3 = wpool.tile([128, O_PER * 4], f16, tag="q3", bufs=2,
                            name=f"q3_{rep}_{i}")
            q3v = q3[:].rearrange("p (o d) -> p o d", d=4)
            nc.vector.tensor_add(q3v, q2v[:, :, 0:4], q2v[:, :, 4:8])
            q4 = wpool.tile([128, O_PER * 2], f16, tag="q4", bufs=2,
                            name=f"q4_{rep}_{i}")
            q4v = q4[:].rearrange("p (o d) -> p o d", d=2)
            nc.vector.tensor_add(q4v, q3v[:, :, 0:2], q3v[:, :, 2:4])
            ot = opool.tile([128, O_PER], f32, tag="ot", name=f"ot{rep}_{i}")
            otv = ot[:].rearrange("p (o d) -> p o d", d=1)
            nc.vector.tensor_add(otv, q4v[:, :, 0:1], q4v[:, :, 1:2])
            nc.vector.tensor_add(ot[:], ot[:], brep[:])
            nc.scalar.dma_start(y[i * BT : (i + 1) * BT, :], ot[:])


def _build_program(n_reps=1):
    f32 = mybir.dt.float32
    f16 = mybir.dt.float16
    nc = bass.Bass()
    x = nc.dram_tensor("x", [BATCH, F_PER], f32, kind="ExternalInput")
    w = nc.dram_tensor("w", [128, F_PER], f16, kind="ExternalInput")
    b = nc.dram_tensor("b", [128, O_PER], f32, kind="ExternalInput")
    y = nc.dram_tensor("y", [BATCH, O_PER], f32, kind="ExternalOutput")
    for rep in range(n_reps):
        with TileContext(nc) as tc:
            _emit_body(nc, tc, x, w, b, y, rep=rep)
    return nc


def _finalize(nc):
    data = _split_multi_waits(nc.to_json_bytes())
    nc.to_json_bytes = lambda: data
    return nc


_CACHED = None


def _get_program():
    global _CACHED
    if _CACHED is None:
        _patch_walrus_flags()
        _CACHED = _finalize(_build_program())
    return _CACHED


def _shard_inputs(x, weight, bias):
    x = np.ascontiguousarray(np.asarray(x, dtype=np.float32))
    weight = np.asarray(weight, dtype=np.float32)
    bias = np.asarray(bias, dtype=np.float32)
    assert x.shape == (BATCH, OUT_DIM * DPC) and weight.shape == (OUT_DIM, OUT_DIM * DPC)
    # Diagonal strip of weight: wd[o, d] = weight[o, o*DPC + d]  [OUT_DIM, DPC]
    w3 = weight.reshape(OUT_DIM, OUT_DIM, DPC)
    wd = w3[np.arange(OUT_DIM), np.arange(OUT_DIM)].astype(np.float16)
    in_maps = []
    for k in range(N_CORES):
        fs = slice(k * F_PER, (k + 1) * F_PER)
        os_ = slice(k * O_PER, (k + 1) * O_PER)
        wk = wd[os_].reshape(1, F_PER)
        bk = bias[os_].reshape(1, O_PER)
        in_maps.append(
            {
                "x": np.ascontiguousarray(x[:, fs]),
                "w": np.ascontiguousarray(np.broadcast_to(wk, (128, F_PER))),
                "b": np.ascontiguousarray(np.broadcast_to(bk, (128, O_PER))),
            }
        )
    return in_maps


def kernel(x, weight, bias):
    nc = _get_program()
    in_maps = _shard_inputs(x, weight, bias)
    res = run_bass_kernel_spmd(nc, in_maps, list(range(N_CORES))).results
    return np.concatenate([res[k]["y"] for k in range(N_CORES)], axis=1)


if __name__ == "__main__":
    rng = np.random.default_rng(0)
    x = rng.standard_normal((BATCH, OUT_DIM * DPC), dtype=np.float32)
    w = rng.standard_normal((OUT_DIM, OUT_DIM * DPC), dtype=np.float32)
    b_ = rng.standard_normal(OUT_DIM).astype(np.float32)
    out = kernel(x, w, b_)
    xb = x.reshape(BATCH, OUT_DIM, DPC)
    wb = np.stack([w[o, o * DPC : (o + 1) * DPC] for o in range(OUT_DIM)])
    exp = np.einsum("bod,od->bo", xb, wb) + b_
    rel = np.linalg.norm(out - exp) / np.linalg.norm(exp)
    print("rel err:", rel)
